# revision 22
# baseline (speedup 1.0000x reference)
"""Trainium2 Bass kernel for nn_MultiHeadDotProductAttention (b=4, L=2048,
d_model=1024, 16 heads x 64 head_dim, additive attention bias, softmax).

Sharding: 8 cores = 2 batch-groups (2 batches each) x 4 head-groups (4 heads
each). Each core computes, for its 2 batches and 4 heads, the full attention
pipeline and an output-projection PARTIAL (summed over its 4 heads); the host
sums the 4 head-group partials per batch and adds the output bias.

Device layout ("T layout"): sequence-length on the free dim, feature dims on
partitions, so no on-device transposes are needed:
  qT,kT: [hd, l]    from  out = wq^T @ xT  (xT transposed on host)
  logitsT[lk, lq] = kT-slices^T @ qT (K=64, two heads row-packed via
                    tile_position into one [128,1024] 2-bank PSUM tile)
  softmax: the additive bias is applied MULTIPLICATIVELY: the host streams
           expb = exp(bias) as bf16 and the device computes
           attn = exp(logits) * expb  (one wide ACT exp from PSUM + one
           bf16 2x-mode DVE multiply). Denominators come for free from an
           all-ones column appended to V in the AV matmul; normalization is
           reciprocal_approx_fast + ones-matmul partition-broadcast + DVE mul.
  out = ctxT^T @ wo with ctxT [hd, lq] directly produced by AV.

Algebraic simplifications vs the reference:
  - bk is dropped entirely: q.(k+bk) adds a per-(b,h,lq) constant to every
    logit in the softmax row, which cancels exactly in softmax.
  - bv is dropped on-device: its contribution is ctx += bv per head, so
    out += sum_h bv_h @ wo_h -- a constant [D] vector added on the host.
  - the 1/sqrt(head_dim) query scale is folded into wq/bq on the host.

P1 (projections) runs weight-stationary with N=512 matmuls; P3 (output
projection) is interleaved into the tail of P2 per 512-row lq slice.
"""

import numpy as np
from contextlib import ExitStack

import ml_dtypes

import concourse.bass as bass
import concourse.mybir as mybir
import concourse.tile as tile
from concourse import bacc
from concourse import bass_utils

F32 = mybir.dt.float32
F32R = mybir.dt.float32r
BF16 = mybir.dt.bfloat16
AF = mybir.ActivationFunctionType

# ---- problem constants (hardcoded per contract) ----
B, L, D = 4, 2048, 1024
H, DH = 16, 64
NB = 2          # batch groups (batches per core = B // NB = 2)
NH = 4          # head groups  (heads per core = H // NH = 4)
BPC = B // NB   # 2 batches per core
HPC = H // NH   # 4 heads per core
PAIRS = HPC // 2
KSUB = D // 128          # 8 contraction subtiles for projections
NQ = 4                   # lq chunks of 512 for attention
NI = 16                  # lk chunks of 128
HD = HPC * DH            # 256 local head dims
HDC = HD // 128          # 2 local hd chunks (= PAIRS)

# P3 partial-output dtype streamed back to the host
OUT_DT = "bf16"

_CACHED = {}


def _build_bass():
    nc = bacc.Bacc("TRN2", target_bir_lowering=False, debug=False, num_devices=8)

    out_dt = F32 if OUT_DT == "f32" else BF16

    # ---- DRAM I/O (per core) ----
    xq_d = nc.dram_tensor("xq_t", [BPC, D, L], BF16, kind="ExternalInput")
    xk_d = nc.dram_tensor("xk_t", [BPC, D, L], BF16, kind="ExternalInput")
    expb_d = nc.dram_tensor("expb_t", [HPC, L, L], BF16, kind="ExternalInput")
    wq_d = nc.dram_tensor("wq", [D, HD], BF16, kind="ExternalInput")
    wk_d = nc.dram_tensor("wk", [D, HD], BF16, kind="ExternalInput")
    wv_d = nc.dram_tensor("wv", [D, HD], BF16, kind="ExternalInput")
    wo_d = nc.dram_tensor("wo", [HD, D], BF16, kind="ExternalInput")
    bq_d = nc.dram_tensor("bq", [HD], F32, kind="ExternalInput")
    out_d = nc.dram_tensor("out_part", [BPC, L, D], out_dt, kind="ExternalOutput")

    with tile.TileContext(nc) as tc, ExitStack() as top:
        # ---- persistent SBUF ----
        pers = top.enter_context(tc.tile_pool(name="pers", bufs=1))
        qT = pers.tile([128, HDC, BPC, L], BF16)
        kT = pers.tile([128, HDC, BPC, L], BF16)
        v = pers.tile([128, NI, BPC, HPC, DH + 1], BF16)
        ctxT = pers.tile([128, HDC, BPC, L], BF16)
        wo_s = pers.tile([128, HDC, D], BF16)
        bq_s = pers.tile([128, HDC], F32)
        nc.sync.dma_start(wo_s[:], wo_d.rearrange("(c p) n -> p c n", p=128))
        nc.sync.dma_start(bq_s[:], bq_d.rearrange("(c p) -> p c", p=128))
        ones_f32 = pers.tile([128, 128], F32)
        nc.vector.memset(ones_f32[:], 1.0)
        # softmax-denominator column of v (column 0 is all-ones, so the
        # denominator row lands on PSUM partition 0 where the DVE reciprocal
        # and the gpsimd partition-broadcast can reach it directly)
        nc.vector.tensor_copy(
            v[:, :, :, :, 0],
            ones_f32[:, 0:NI * BPC * HPC].rearrange(
                "p (a b c) -> p a b c", a=NI, b=BPC
            ),
        )

        # ---- P1: projections (weight-stationary, N=512) ----
        with ExitStack() as p1:
            wpool = p1.enter_context(tc.tile_pool(name="wqkv", bufs=1))
            wq_s = wpool.tile([128, KSUB, HD], BF16)
            wk_s = wpool.tile([128, KSUB, HD], BF16)
            wv_s = wpool.tile([128, KSUB, HD], BF16)
            nc.sync.dma_start(wq_s[:], wq_d.rearrange("(k p) n -> p k n", p=128))
            nc.sync.dma_start(wk_s[:], wk_d.rearrange("(k p) n -> p k n", p=128))
            nc.sync.dma_start(wv_s[:], wv_d.rearrange("(k p) n -> p k n", p=128))

            xpool = p1.enter_context(tc.tile_pool(name="xs", bufs=2))
            pp = p1.enter_context(tc.tile_pool(name="pqk", bufs=1, space="PSUM"))
            psv = p1.enter_context(tc.tile_pool(name="psv", bufs=3, space="PSUM"))

            def qk_gen(xq_t, xk_t, b, h2):
                # q then k: 4-bank groups accumulating over KSUB with a
                # stationary weight slice serving both 512-wide chunks; each
                # bank drains (ACT for q with bias, DVE for k) right after
                # its last accumulation matmul.
                for which in range(2):
                    w_s = wq_s if which == 0 else wk_s
                    x_t = xq_t if which == 0 else xk_t
                    ps = {}
                    for m in range(HDC):
                        for c in range(2):
                            ps[m, c] = pp.tile(
                                [128, 512], F32, tag=f"p{m}{c}",
                                name=f"ps{m}{c}",
                            )
                    for k in range(KSUB):
                        for m in range(HDC):
                            msl = slice(m * 128, (m + 1) * 128)
                            for c in range(2):
                                csl = slice(c * 512, (c + 1) * 512)
                                nc.tensor.matmul(
                                    ps[m, c][:], w_s[:, k, msl], x_t[:, k, csl],
                                    start=(k == 0), stop=(k == KSUB - 1),
                                )
                                if k == KSUB - 1:
                                    osl = slice(
                                        h2 * 1024 + c * 512,
                                        h2 * 1024 + (c + 1) * 512,
                                    )
                                    if which == 0:
                                        nc.scalar.activation(
                                            qT[:, m, b, osl], ps[m, c][:],
                                            AF.Identity, bias=bq_s[:, m:m + 1],
                                        )
                                    else:
                                        nc.vector.tensor_copy(
                                            kT[:, m, b, osl], ps[m, c][:]
                                        )
                                yield

            def v_gen(xk_t, b, h2):
                # v: out[lk-sub(128), hd(256)] = xT-slices^T @ wv
                for s in range(8):
                    si = h2 * 8 + s
                    pv = psv.tile([128, HD], F32, tag="pv", name="pv")
                    for k in range(KSUB):
                        nc.tensor.matmul(
                            pv[:], xk_t[:, k, s * 128:(s + 1) * 128],
                            wv_s[:, k, :],
                            start=(k == 0), stop=(k == KSUB - 1),
                        )
                        if k == KSUB - 1:
                            nc.vector.tensor_copy(
                                v[:, si, b, :, 1:DH + 1],
                                pv[:].rearrange("p (h d) -> p h d", h=HPC),
                            )
                        yield

            from itertools import zip_longest

            for b in range(BPC):
                xqr = xq_d[b].rearrange("(k p) l -> p k l", p=128)
                xkr = xk_d[b].rearrange("(k p) l -> p k l", p=128)
                for h2 in range(2):
                    hsl = slice(h2 * 1024, (h2 + 1) * 1024)
                    xq_t = xpool.tile([128, KSUB, 1024], BF16, tag="xq")
                    xk_t = xpool.tile([128, KSUB, 1024], BF16, tag="xk")
                    # per-k-subtile DMAs: the k=0 matmuls start as soon as
                    # the first chunk lands instead of waiting for 4MB
                    for k in range(KSUB):
                        nc.sync.dma_start(xq_t[:, k, :], xqr[:, k, hsl])
                        nc.sync.dma_start(xk_t[:, k, :], xkr[:, k, hsl])

                    # interleave v matmuls 1:1 between q/k matmuls so the
                    # v LDWEIGHTS loads hide under the longer N=512 matmuls
                    for _ in zip_longest(
                        qk_gen(xq_t, xk_t, b, h2), v_gen(xk_t, b, h2)
                    ):
                        pass

        # ---- P2: attention (+ P3 output projection interleaved) ----
        # Software-pipelined emission: each half-block (p, n, b) runs a
        # 16-step i-loop (logits pair -> wide exp -> expb mul -> AV); the
        # PREVIOUS half-block's normalize and pending P3 tile-groups are
        # emitted INSIDE the i-loop so they never head-of-line-block the PE
        # queue at half-block boundaries (which starves the Scalar engine).
        with ExitStack() as p2:
            ebpool = p2.enter_context(tc.tile_pool(name="ebb", bufs=2))
            epool = p2.enter_context(tc.tile_pool(name="expb", bufs=4))
            apool = p2.enter_context(tc.tile_pool(name="attnb", bufs=4))
            rpool = p2.enter_context(tc.tile_pool(name="recip", bufs=2))
            scpool = p2.enter_context(tc.tile_pool(name="scsh", bufs=2))
            opool = p2.enter_context(tc.tile_pool(name="outb", bufs=4))
            psav = p2.enter_context(tc.tile_pool(name="psav", bufs=2, space="PSUM"))
            pslg = p2.enter_context(tc.tile_pool(name="pslg", bufs=2, space="PSUM"))

            def emit_normalize(p, n, b, av):
                # normalize -> ctxT: denominator sits on PSUM partition 0;
                # reciprocal it on the DVE, broadcast across partitions on
                # the (otherwise idle) gpsimd engine, scale, and DMA the
                # 64 ctx rows into their ctxT partition slot. No PE work.
                nsl = slice(n * 512, (n + 1) * 512)
                for hl in range(2):
                    rcp = rpool.tile([1, 512], F32, tag="rcp", name="rcp")
                    nc.vector.reciprocal_approx_fast(rcp[:], av[hl][0:1, :])
                    rep = rpool.tile([65, 512], F32, tag="rep", name="rep")
                    nc.gpsimd.partition_broadcast(rep[:], rcp[0:1, :])
                    sc = scpool.tile([65, 512], BF16, tag="sc", name="sc")
                    # row 0 computes den*recip(den); only rows 1..64 (the 64
                    # ctx rows) are shipped. PSUM partition bases must be
                    # 32-aligned, so the mul spans [0:65].
                    nc.vector.tensor_mul(
                        sc[0:65, :], av[hl][0:DH + 1, :], rep[0:65, :]
                    )
                    nc.sync.dma_start(
                        ctxT[hl * 64:(hl + 1) * 64, p, b, nsl], sc[1:65, :]
                    )

            def emit_p3_unit(b, m, nn):
                msl = slice(m * 128, (m + 1) * 128)
                osl = slice(nn * 512, (nn + 1) * 512)
                po = pslg.tile([128, 512], F32, tag="lgp", name="po")
                for kc in range(HDC):
                    nc.tensor.matmul(
                        po[:], ctxT[:, kc, b, msl], wo_s[:, kc, osl],
                        start=(kc == 0), stop=(kc == HDC - 1),
                    )
                ot = opool.tile([128, 512], out_dt, tag="ot", name="ot")
                # split the PSUM->SBUF drains between ACT and DVE
                if (m + nn) % 2 == 0:
                    nc.scalar.copy(ot[:], po[:])
                else:
                    nc.vector.tensor_copy(ot[:], po[:])
                nc.sync.dma_start(out_d[b, msl, osl], ot[:])

            def emit_eb_dmas(p, n, i, eb_store):
                isl = slice(i * 128, (i + 1) * 128)
                nsl = slice(n * 512, (n + 1) * 512)
                ebt = ebpool.tile([128, 1024], BF16, tag=f"eb{i}", name=f"eb{i}")
                nc.sync.dma_start(
                    ebt.rearrange("p (h n) -> p h n", h=2),
                    expb_d[2 * p:2 * p + 2, isl, nsl].rearrange(
                        "h p n -> p h n"
                    ),
                )
                eb_store[i] = ebt

            blocks = [(p, n) for p in range(PAIRS) for n in range(NQ)]
            eb_cur, eb_nxt = {}, {}
            for i in range(NI):
                emit_eb_dmas(blocks[0][0], blocks[0][1], i, eb_cur)

            pending_norm = None   # (p, n, b, av) awaiting emission
            pending_av = None     # (av, i, b, p, at) -- AV matmuls delayed 1 step
            p3_queue = []         # (b, m, nn) output-projection units

            def emit_pending_av():
                nonlocal pending_av
                if pending_av is None:
                    return
                pav, pi, pb, pp_, pat = pending_av
                for hl in range(2):
                    nc.tensor.matmul(
                        pav[hl][0:DH + 1, :],
                        v[:, pi, pb, 2 * pp_ + hl, :],
                        pat[:, hl * 512:(hl + 1) * 512],
                        start=(pi == 0), stop=(pi == NI - 1),
                    )
                pending_av = None

            for bi, (p, n) in enumerate(blocks):
                nsl = slice(n * 512, (n + 1) * 512)
                for b in range(BPC):
                    av = {}
                    for hl in range(2):
                        av[hl] = psav.tile(
                            [128, 512], F32, tag=f"av{hl}", name=f"av{hl}"
                        )
                    for i in range(NI):
                        isl = slice(i * 128, (i + 1) * 128)
                        lgp = pslg.tile([128, 1024], F32, tag="lgp", name="lgp")
                        for hl in range(2):
                            rsl = slice(hl * 64, (hl + 1) * 64)
                            nc.tensor.matmul(
                                lgp[:, hl * 512:(hl + 1) * 512],
                                kT[rsl, p, b, isl],
                                qT[rsl, p, b, nsl],
                                start=True, stop=True,
                                tile_position=(hl * 64, 0),
                            )
                        et = epool.tile([128, 1024], BF16, tag="exp", name="et")
                        nc.scalar.activation(et[:], lgp[:], AF.Exp)
                        at = apool.tile([128, 1024], BF16, tag="attn", name="at")
                        nc.vector.tensor_mul(at[:], et[:], eb_cur[i][:])
                        # AV matmuls run one step behind so they never
                        # head-of-line-block the next logits matmul on the
                        # exp->mul dependency chain.
                        emit_pending_av()
                        pending_av = (av, i, b, p, at)
                        # pipelined epilogue work from the previous half-block
                        if i == 1 and pending_norm is not None:
                            emit_normalize(*pending_norm)
                            pn_p, pn_n, pn_b = pending_norm[0], pending_norm[1], pending_norm[2]
                            if pn_p == PAIRS - 1 and pn_b == BPC - 1:
                                p3_queue.extend(
                                    (bb, m, nn)
                                    for bb in range(BPC)
                                    for m in range(4 * pn_n, 4 * pn_n + 4)
                                    for nn in range(D // 512)
                                )
                            pending_norm = None
                        if p3_queue and i >= 2 and i % 2 == 0:
                            emit_p3_unit(*p3_queue.pop(0))
                        # prefetch next block's expb tiles (2 per step, b==1)
                        if b == BPC - 1 and bi + 1 < len(blocks):
                            np_, nn_ = blocks[bi + 1]
                            for j in (2 * i, 2 * i + 1):
                                if j < NI:
                                    emit_eb_dmas(np_, nn_, j, eb_nxt)
                    pending_norm = (p, n, b, av)
                eb_cur, eb_nxt = eb_nxt, {}

            # drain the tail
            emit_pending_av()
            if pending_norm is not None:
                emit_normalize(*pending_norm)
                pn_n = pending_norm[1]
                p3_queue.extend(
                    (bb, m, nn)
                    for bb in range(BPC)
                    for m in range(4 * pn_n, 4 * pn_n + 4)
                    for nn in range(D // 512)
                )
            for unit in p3_queue:
                emit_p3_unit(*unit)

    nc.compile()
    return nc


def make_in_maps(inputs_q, inputs_kv, bias, wq, bq, wk, bk, wv, bv, wo, bo):
    inputs_q = np.asarray(inputs_q, np.float32)
    inputs_kv = np.asarray(inputs_kv, np.float32)
    bias = np.asarray(bias, np.float32)
    wq = np.asarray(wq, np.float32).reshape(D, H * DH)
    wv = np.asarray(wv, np.float32).reshape(D, H * DH)
    wk = np.asarray(wk, np.float32).reshape(D, H * DH)
    bq = np.asarray(bq, np.float32).reshape(H * DH)
    wo = np.asarray(wo, np.float32).reshape(H * DH, D)

    # fold the 1/sqrt(head_dim) query scaling into wq/bq
    s = 1.0 / np.sqrt(DH)
    wq = wq * s
    bq = bq * s

    # host-side layout marshalling for the chosen sharding
    xqT = np.ascontiguousarray(inputs_q.transpose(0, 2, 1)).astype(
        ml_dtypes.bfloat16
    )
    xkT = np.ascontiguousarray(inputs_kv.transpose(0, 2, 1)).astype(
        ml_dtypes.bfloat16
    )
    # multiplicative attention bias, pre-transposed: expbT[h, lk, lq]
    expbT = np.exp(bias[0].transpose(0, 2, 1)).astype(ml_dtypes.bfloat16)

    in_maps = []
    for bg in range(NB):
        bsl = slice(bg * BPC, (bg + 1) * BPC)
        for hg in range(NH):
            hsl = slice(hg * HPC, (hg + 1) * HPC)
            csl = slice(hg * HD, (hg + 1) * HD)
            in_maps.append(
                {
                    "xq_t": xqT[bsl],
                    "xk_t": xkT[bsl],
                    "expb_t": np.ascontiguousarray(expbT[hsl]),
                    "wq": np.ascontiguousarray(wq[:, csl]).astype(ml_dtypes.bfloat16),
                    "wk": np.ascontiguousarray(wk[:, csl]).astype(ml_dtypes.bfloat16),
                    "wv": np.ascontiguousarray(wv[:, csl]).astype(ml_dtypes.bfloat16),
                    "wo": np.ascontiguousarray(wo[csl, :]).astype(ml_dtypes.bfloat16),
                    "bq": np.ascontiguousarray(bq[csl]),
                }
            )
    return in_maps


def assemble(results, bv, wo, bo):
    """Sum per-head-group output partials; bv's contribution to the output is
    the constant vector sum_h bv_h @ wo_h, added here alongside bo."""
    bv = np.asarray(bv, np.float32).reshape(H * DH)
    wo = np.asarray(wo, np.float32).reshape(H * DH, D)
    bo = np.asarray(bo, np.float32)
    out = np.zeros((B, L, D), np.float32)
    for bg in range(NB):
        for hg in range(NH):
            out[bg * BPC:(bg + 1) * BPC] += np.asarray(
                results[bg * NH + hg]["out_part"], np.float32
            )
    out += bo + bv @ wo
    return out


def get_nc():
    if "nc" not in _CACHED:
        _CACHED["nc"] = _build_bass()
    return _CACHED["nc"]


def kernel(inputs_q, inputs_kv, bias, wq, bq, wk, bk, wv, bv, wo, bo):
    in_maps = make_in_maps(
        inputs_q, inputs_kv, bias, wq, bq, wk, bk, wv, bv, wo, bo
    )
    res = bass_utils.run_bass_kernel_spmd(
        get_nc(), in_maps, core_ids=list(range(8))
    )
    return assemble(res.results, bv, wo, bo)
